# revision 12
# baseline (speedup 1.0000x reference)
"""Causal self-attention Trainium2 Bass kernel (8-core SPMD).

Problem: x[4,2048,1024] -> CausalSelfAttention(n_head=16) -> [4,2048,1024], f32.

Sharding: 8 cores = 4 batches x 2 head-groups (8 heads each). No collectives:
each core computes its head-group's partial output projection; the host sums
the two partials per batch and adds the folded bias.

Per-core dataflow (all matmuls fp32r, transpose-free):
  phase 1: qT = (w_q*s).T @ xT + b_q*s   [512,2048]  feature-major
           kT = w_k.T @ xT               [512,2048]  (b_k dropped: softmax
                                                      row-shift invariance)
           V  = xT.T @ w_v               [2048,512]  token-major, augmented
                                                     with a ones column
  phase 2: per (q-chunk j, head pair): S^T = kT.T-slice @ qT-slice (k on
           partitions), additive causal mask on diagonal 128x128 squares,
           P = exp(S^T) (no max subtraction; scores are O(3)), PV^T
           accumulated over k-tiles with the ones column producing row sums r
           in row 64; normalize via r^-1 broadcast (DRAM bounce), then
           Y^T += w_pr.T @ OUT^T per q-chunk.
Host folds b_v and b_proj into one constant vector: b_v @ w_proj + b_proj
(softmax rows sum to 1, so the V bias passes through exactly).
"""

import sys

if "/opt/trn_rl_repo" not in sys.path:
    sys.path.insert(0, "/opt/trn_rl_repo")

from contextlib import ExitStack

import numpy as np

import concourse.bass as bass
import concourse.tile as tile
from concourse import bacc, mybir
from concourse.bass_utils import run_bass_kernel_spmd

F32 = mybir.dt.float32
F32R = mybir.dt.float32r
AFT = mybir.ActivationFunctionType

C = 1024          # n_embd
T = 2048          # seq len
NB = 4            # batch
NHEAD = 16
HD = 64           # head dim
HPG = 8           # heads per group (per core)
GC = HPG * HD     # 512 features per group
NKC = C // 128    # 8 contraction tiles over C
NCHUNK = 256      # phase-1 token chunk
NJ = T // 512     # 4 q-chunks of 512
MASK_NEG = -1.0e4


def build_kernel(nc: bass.Bass):
    xT = nc.dram_tensor("xT", [C, T], F32R, kind="ExternalInput")
    w_qk = nc.dram_tensor("w_qk", [C, 2 * GC], F32R, kind="ExternalInput")
    w_v = nc.dram_tensor("w_v", [C, GC], F32R, kind="ExternalInput")
    b_q = nc.dram_tensor("b_q", [GC], F32, kind="ExternalInput")
    w_pr = nc.dram_tensor("w_pr", [GC, C], F32R, kind="ExternalInput")
    mask128 = nc.dram_tensor("mask128", [128, 128], F32, kind="ExternalInput")
    yT = nc.dram_tensor("yT", [C, T], F32, kind="ExternalOutput")
    r_bounce = nc.dram_tensor("r_bounce", [NJ, HPG, 512], F32)

    with tile.TileContext(nc) as tc, ExitStack() as ctx:
        persist = ctx.enter_context(tc.tile_pool(name="persist", bufs=1))
        # feature-major q/k: [partition, head-pair tile, token];
        # pair tile t holds head 2t on partitions 0:64, head 2t+1 on 64:128
        qT_s = persist.tile([128, 4, T], F32R, tag="qT")
        kT_s = persist.tile([128, 4, T], F32R, tag="kT")
        # token-major V: [token%128, token//128, head, hd+1]; col 64 = ones
        v_s = persist.tile([128, T // 128, HPG, HD + 1], F32R, tag="v")
        ones_s = persist.tile([128, T // 128 * HPG], F32, tag="ones")
        nc.vector.memset(ones_s, 1.0)
        nc.vector.tensor_copy(
            v_s[:, :, :, HD : HD + 1],
            ones_s.rearrange("p (a b) -> p a b", a=T // 128),
        )

        # ---------------- phase 1: projections ----------------
        with ExitStack() as p1:
            consts1 = p1.enter_context(tc.tile_pool(name="consts1", bufs=1))
            xpool = p1.enter_context(tc.tile_pool(name="xchunk", bufs=2))
            psum1 = p1.enter_context(
                tc.tile_pool(name="psum1", bufs=4, space="PSUM")
            )

            w_qk_s = consts1.tile([128, NKC, 2 * GC], F32R, tag="wqk")
            nc.sync.dma_start(
                out=w_qk_s, in_=w_qk.rearrange("(kc p) n -> p kc n", p=128)
            )
            w_v_s = consts1.tile([128, NKC, GC], F32R, tag="wv")
            nc.sync.dma_start(
                out=w_v_s, in_=w_v.rearrange("(kc p) n -> p kc n", p=128)
            )
            b_q_s = consts1.tile([128, 4], F32, tag="bq")
            nc.sync.dma_start(out=b_q_s, in_=b_q.rearrange("(m p) -> p m", p=128))

            for n in range(T // NCHUNK):
                xc = xpool.tile([128, NKC, NCHUNK], F32R, tag="xc")
                for kc in range(NKC):
                    nc.sync.dma_start(
                        out=xc[:, kc, :],
                        in_=xT[kc * 128 : (kc + 1) * 128,
                              n * NCHUNK : (n + 1) * NCHUNK],
                    )
                tok = slice(n * NCHUNK, (n + 1) * NCHUNK)
                # q and k feature tiles (m 0..3 -> q pairs, 4..7 -> k pairs)
                for m in range(8):
                    ps = psum1.tile([128, NCHUNK], F32, tag="ps1")
                    for kc in range(NKC):
                        nc.tensor.matmul(
                            ps,
                            lhsT=w_qk_s[:, kc, m * 128 : (m + 1) * 128],
                            rhs=xc[:, kc, :],
                            start=(kc == 0),
                            stop=(kc == NKC - 1),
                        )
                    if m < 4:
                        nc.scalar.activation(
                            qT_s[:, m, tok], ps, AFT.Identity,
                            bias=b_q_s[:, m : m + 1], scale=1.0,
                        )
                    else:
                        nc.vector.tensor_copy(kT_s[:, m - 4, tok], ps)
                # V tiles: token-major, two 128-token tiles per chunk
                for mv in range(NCHUNK // 128):
                    m16 = (n * NCHUNK) // 128 + mv
                    ps = psum1.tile([128, GC], F32, tag="ps1v")
                    for kc in range(NKC):
                        nc.tensor.matmul(
                            ps,
                            lhsT=xc[:, kc, mv * 128 : (mv + 1) * 128],
                            rhs=w_v_s[:, kc, :],
                            start=(kc == 0),
                            stop=(kc == NKC - 1),
                        )
                    nc.vector.tensor_copy(
                        v_s[:, m16, :, 0:HD],
                        ps.rearrange("p (h d) -> p h d", h=HPG),
                    )

        # ---------------- phase 2: attention + projection ----------------
        with ExitStack() as p2:
            consts2 = p2.enter_context(tc.tile_pool(name="consts2", bufs=1))
            ppool = p2.enter_context(tc.tile_pool(name="ppool", bufs=3))
            rpool = p2.enter_context(tc.tile_pool(name="rpool", bufs=2))
            prhs_pool = p2.enter_context(tc.tile_pool(name="prhs", bufs=2))
            ypool = p2.enter_context(tc.tile_pool(name="ypool", bufs=3))
            psum_s = p2.enter_context(
                tc.tile_pool(name="psum_s", bufs=2, space="PSUM")
            )
            psum_o = p2.enter_context(
                tc.tile_pool(name="psum_o", bufs=1, space="PSUM")
            )
            psum_y = p2.enter_context(
                tc.tile_pool(name="psum_y", bufs=2, space="PSUM")
            )

            w_pr_s = consts2.tile([128, 4, C], F32R, tag="wpr")
            nc.sync.dma_start(
                out=w_pr_s, in_=w_pr.rearrange("(kc p) n -> p kc n", p=128)
            )
            mask_s = consts2.tile([128, 128], F32, tag="mask")
            nc.sync.dma_start(out=mask_s, in_=mask128[:, :])

            for j in range(NJ):
                prhs = prhs_pool.tile([128, 4, 512], F32R, tag="prhs")
                nkc = 4 * j + 4
                for t in range(4):  # head pairs (2t, 2t+1)
                    outs = [
                        psum_o.tile([128, 512], F32, tag=f"out{ab}",
                                    name=f"out{ab}")
                        for ab in range(2)
                    ]
                    for kc in range(nkc):
                        c = kc - 4 * j  # >=0 -> diagonal band tile
                        lo = 128 * c if c > 0 else 0
                        for ab in range(2):
                            part = slice(ab * 64, ab * 64 + 64)
                            st = psum_s.tile([128, 512], F32, tag=f"s{ab}")
                            nc.tensor.matmul(
                                st[:, lo:],
                                lhsT=
                                    kT_s[part, t, kc * 128 : (kc + 1) * 128]
                                ,
                                rhs=qT_s[part, t, j * 512 + lo : (j + 1) * 512],
                                start=True,
                                stop=True,
                                tile_position=(ab * 64, 0),
                            )
                            if c >= 0:
                                nc.vector.tensor_add(
                                    st[:, lo : lo + 128],
                                    st[:, lo : lo + 128],
                                    mask_s,
                                )
                            pt = ppool.tile([128, 512], F32R, tag=f"p{ab}")
                            nc.scalar.activation(
                                pt[:, lo:], st[:, lo:], AFT.Exp
                            )
                            nc.tensor.matmul(
                                outs[ab][0 : HD + 1, lo:],
                                lhsT=v_s[:, kc, 2 * t + ab, :],
                                rhs=pt[:, lo:],
                                start=(kc == 0),
                                stop=(kc == nkc - 1),
                            )
                    for ab in range(2):
                        part = slice(ab * 64, ab * 64 + 64)
                        nc.vector.tensor_copy(prhs[part, t, :], outs[ab][0:HD, :])
                        r_h = rpool.tile([1, 512], F32, tag="rh",
                                         name="rh", bufs=4)
                        nc.vector.tensor_copy(r_h, outs[ab][HD : HD + 1, :])
                        nc.sync.dma_start(
                            out=r_bounce[j, 2 * t + ab], in_=r_h
                        )
                # normalize: r^-1 broadcast along the 64 hd-partitions of
                # each head (DRAM bounce for the partition-broadcast)
                rb = rpool.tile([128, 4, 512], F32, tag="rb")
                for h in range(HPG):
                    src = r_bounce[j, h, :]
                    bsrc = bass.AP(
                        tensor=src.tensor,
                        offset=src.offset,
                        ap=[[0, 64]] + list(src.ap),
                    )
                    nc.sync.dma_start(
                        out=rb[(h % 2) * 64 : (h % 2) * 64 + 64, h // 2, :],
                        in_=bsrc,
                    )
                nc.vector.reciprocal(rb, rb)
                for t in range(4):
                    nc.vector.tensor_mul(prhs[:, t, :], prhs[:, t, :], rb[:, t, :])
                # output projection for this q-chunk
                for mp in range(8):
                    ps = psum_y.tile([128, 512], F32, tag="psy")
                    for kcp in range(4):
                        nc.tensor.matmul(
                            ps,
                            lhsT=w_pr_s[:, kcp, mp * 128 : (mp + 1) * 128],
                            rhs=prhs[:, kcp, :],
                            start=(kcp == 0),
                            stop=(kcp == 3),
                        )
                    yb = ypool.tile([128, 512], F32, tag="yb")
                    nc.vector.tensor_copy(yb, ps)
                    nc.sync.dma_start(
                        out=yT[mp * 128 : (mp + 1) * 128,
                               j * 512 : (j + 1) * 512],
                        in_=yb,
                    )
    return nc


def round_fp32r(a):
    """Round fp32 to fp32r (11-bit mantissa), matching walrus fp32_to_fp32r."""
    a = np.ascontiguousarray(a, dtype=np.float32)
    bits = a.view(np.uint32)
    out = ((bits.astype(np.uint64) + 0x800) & 0xFFFFF000).astype(np.uint32)
    return out.view(np.float32)


def make_core_inputs(x, w_attn, b_attn, w_proj):
    """Per-core input dicts; core index = batch*2 + head_group."""
    scale = np.float32(1.0 / np.sqrt(HD))
    mask = np.where(
        np.arange(128)[:, None] <= np.arange(128)[None, :], 0.0, MASK_NEG
    ).astype(np.float32)
    in_maps = []
    for b in range(NB):
        xTb = round_fp32r(x[b].T)
        for g in range(2):
            cols = slice(g * GC, (g + 1) * GC)
            w_qk = np.concatenate(
                [w_attn[:, cols] * scale, w_attn[:, C:][:, cols]], axis=1
            ).astype(np.float32)
            in_maps.append(
                {
                    "xT": xTb,
                    "w_qk": round_fp32r(w_qk),
                    "w_v": round_fp32r(w_attn[:, 2 * C :][:, cols]),
                    "b_q": np.ascontiguousarray(
                        b_attn[cols] * scale, dtype=np.float32
                    ),
                    "w_pr": round_fp32r(w_proj[cols, :]),
                    "mask128": mask,
                }
            )
    return in_maps


_NC_CACHE = None


def get_nc():
    global _NC_CACHE
    if _NC_CACHE is None:
        nc = bacc.Bacc("TRN2", target_bir_lowering=False, debug=False)
        build_kernel(nc)
        nc.compile()
        _NC_CACHE = nc
    return _NC_CACHE


def kernel(x, w_attn, b_attn, w_proj, b_proj, _want_trace=False):
    x = np.asarray(x, dtype=np.float32)
    w_attn = np.asarray(w_attn, dtype=np.float32)
    b_attn = np.asarray(b_attn, dtype=np.float32)
    w_proj = np.asarray(w_proj, dtype=np.float32)
    b_proj = np.asarray(b_proj, dtype=np.float32)

    nc = get_nc()
    in_maps = make_core_inputs(x, w_attn, b_attn, w_proj)
    res = run_bass_kernel_spmd(
        nc, in_maps, list(range(8)), trace=_want_trace
    )
    bias_total = (b_attn[2 * C :] @ w_proj + b_proj).astype(np.float32)
    out = np.empty((NB, T, C), np.float32)
    for b in range(NB):
        out[b] = (
            res.results[2 * b]["yT"].T
            + res.results[2 * b + 1]["yT"].T
            + bias_total[None, :]
        )
    if _want_trace:
        return out, res
    return out


# revision 17
# speedup vs baseline: 1.1683x; 1.1683x over previous
"""Causal self-attention Trainium2 Bass kernel (8-core SPMD).

Problem: x[4,2048,1024] -> CausalSelfAttention(n_head=16) -> [4,2048,1024], f32.

Sharding: 8 cores = 4 batches x 2 head-groups (8 heads each). No collectives:
each core computes its head-group's partial output projection; the host sums
the two partials per batch and adds the folded bias.

Per-core dataflow (all matmuls fp32r, transpose-free):
  phase 1: qT = (w_q*s).T @ xT + b_q*s   [512,2048]  feature-major
           kT = w_k.T @ xT               [512,2048]  (b_k dropped: softmax
                                                      row-shift invariance)
           V  = xT.T @ w_v               [2048,512]  token-major, augmented
                                                     with a ones column
  phase 2: per (q-chunk j, head pair): S^T = kT.T-slice @ qT-slice (k on
           partitions), additive causal mask on diagonal 128x128 squares,
           P = exp(S^T) (no max subtraction; scores are O(3)), PV^T
           accumulated over k-tiles with the ones column producing row sums r
           in row 64; normalize via r^-1 broadcast (DRAM bounce), then
           Y^T += w_pr.T @ OUT^T per q-chunk.
Host folds b_v and b_proj into one constant vector: b_v @ w_proj + b_proj
(softmax rows sum to 1, so the V bias passes through exactly).
"""

import sys

if "/opt/trn_rl_repo" not in sys.path:
    sys.path.insert(0, "/opt/trn_rl_repo")

from contextlib import ExitStack

import numpy as np

import concourse.bass as bass
import concourse.tile as tile
from concourse import bacc, mybir
from concourse.bass_utils import run_bass_kernel_spmd

F32 = mybir.dt.float32
F32R = mybir.dt.float32r
AFT = mybir.ActivationFunctionType

C = 1024          # n_embd
T = 2048          # seq len
NB = 4            # batch
NHEAD = 16
HD = 64           # head dim
HPG = 8           # heads per group (per core)
GC = HPG * HD     # 512 features per group
NKC = C // 128    # 8 contraction tiles over C
NCHUNK = 512      # phase-1 token chunk
NJ = T // 512     # 4 q-chunks of 512
MASK_NEG = -1.0e4


def bcast(ap, n, axis):
    """Insert a step-0 (broadcast) dim of size n at free-dim position axis."""
    steps = list(ap.ap)
    steps.insert(axis, [0, n])
    return bass.AP(tensor=ap.tensor, offset=ap.offset, ap=steps)


def build_kernel(nc: bass.Bass):
    xT = nc.dram_tensor("xT", [C, T], F32R, kind="ExternalInput")
    w_qk = nc.dram_tensor("w_qk", [C, 2 * GC], F32R, kind="ExternalInput")
    w_v = nc.dram_tensor("w_v", [C, GC], F32R, kind="ExternalInput")
    b_q = nc.dram_tensor("b_q", [GC], F32, kind="ExternalInput")
    w_pr = nc.dram_tensor("w_pr", [GC, C], F32R, kind="ExternalInput")
    mask128 = nc.dram_tensor("mask128", [128, 128], F32, kind="ExternalInput")
    yT = nc.dram_tensor("yT", [C, T], F32, kind="ExternalOutput")
    r_bounce = nc.dram_tensor("r_bounce", [NJ, HPG, 512], F32)

    with tile.TileContext(nc) as tc, ExitStack() as ctx:
        persist = ctx.enter_context(tc.tile_pool(name="persist", bufs=1))
        # feature-major q/k: [partition, head-pair tile, token];
        # pair tile t holds head 2t on partitions 0:64, head 2t+1 on 64:128
        qT_s = persist.tile([128, 4, T], F32R, tag="qT")
        kT_s = persist.tile([128, 4, T], F32R, tag="kT")
        # token-major V: [token%128, token//128, head, hd+1]; col 64 = ones
        v_s = persist.tile([128, T // 128, HPG, HD + 1], F32R, tag="v")
        ones_s = persist.tile([128, T // 128 * HPG], F32, tag="ones")
        nc.vector.memset(ones_s, 1.0)
        nc.vector.tensor_copy(
            v_s[:, :, :, HD : HD + 1],
            ones_s.rearrange("p (a b) -> p a b", a=T // 128),
        )

        # ---------------- phase 1: projections ----------------
        with ExitStack() as p1:
            consts1 = p1.enter_context(tc.tile_pool(name="consts1", bufs=1))
            xpool = p1.enter_context(tc.tile_pool(name="xchunk", bufs=2))
            psum1 = p1.enter_context(
                tc.tile_pool(name="psum1", bufs=4, space="PSUM")
            )

            w_qk_s = consts1.tile([128, NKC, 2 * GC], F32R, tag="wqk")
            nc.sync.dma_start(
                out=w_qk_s, in_=w_qk.rearrange("(kc p) n -> p kc n", p=128)
            )
            w_v_s = consts1.tile([128, NKC, GC], F32R, tag="wv")
            nc.sync.dma_start(
                out=w_v_s, in_=w_v.rearrange("(kc p) n -> p kc n", p=128)
            )
            b_q_s = consts1.tile([128, 4], F32, tag="bq")
            nc.sync.dma_start(out=b_q_s, in_=b_q.rearrange("(m p) -> p m", p=128))

            xT_r = xT.rearrange("(kc p) t -> p kc t", p=128)
            for n in range(T // NCHUNK):
                xc = xpool.tile([128, NKC, NCHUNK], F32R, tag="xc")
                for kc in range(NKC):
                    nc.sync.dma_start(
                        out=xc[:, kc, :],
                        in_=xT_r[:, kc, n * NCHUNK : (n + 1) * NCHUNK],
                    )
                tok = slice(n * NCHUNK, (n + 1) * NCHUNK)
                # q and k feature tiles (m 0..3 -> q pairs, 4..7 -> k pairs)
                for m in range(8):
                    ps = psum1.tile([128, NCHUNK], F32, tag="ps1")
                    for kc in range(NKC):
                        nc.tensor.matmul(
                            ps,
                            lhsT=w_qk_s[:, kc, m * 128 : (m + 1) * 128],
                            rhs=xc[:, kc, :],
                            start=(kc == 0),
                            stop=(kc == NKC - 1),
                        )
                    if m < 4:
                        nc.scalar.activation(
                            qT_s[:, m, tok], ps, AFT.Identity,
                            bias=b_q_s[:, m : m + 1], scale=1.0,
                        )
                    else:
                        nc.vector.tensor_copy(kT_s[:, m - 4, tok], ps)
            # V pass: token-major tiles, xT re-streamed in 128-token slices
            for m16 in range(T // 128):
                xv = xpool.tile([128, NKC, 128], F32R, tag="xc")
                nc.sync.dma_start(
                    out=xv, in_=xT_r[:, :, m16 * 128 : (m16 + 1) * 128]
                )
                ps = psum1.tile([128, GC], F32, tag="ps1v")
                for kc in range(NKC):
                    nc.tensor.matmul(
                        ps,
                        lhsT=xv[:, kc, :],
                        rhs=w_v_s[:, kc, :],
                        start=(kc == 0),
                        stop=(kc == NKC - 1),
                    )
                nc.vector.tensor_copy(
                    v_s[:, m16, :, 0:HD],
                    ps.rearrange("p (h d) -> p h d", h=HPG),
                )

        # ---------------- phase 2: attention + projection ----------------
        with ExitStack() as p2:
            consts2 = p2.enter_context(tc.tile_pool(name="consts2", bufs=1))
            ppool = p2.enter_context(tc.tile_pool(name="ppool", bufs=3))
            rpool = p2.enter_context(tc.tile_pool(name="rpool", bufs=2))
            prhs_pool = p2.enter_context(tc.tile_pool(name="prhs", bufs=2))
            ypool = p2.enter_context(tc.tile_pool(name="ypool", bufs=3))
            psum_s = p2.enter_context(
                tc.tile_pool(name="psum_s", bufs=2, space="PSUM")
            )
            psum_o = p2.enter_context(
                tc.tile_pool(name="psum_o", bufs=1, space="PSUM")
            )
            psum_y = p2.enter_context(
                tc.tile_pool(name="psum_y", bufs=2, space="PSUM")
            )

            w_pr_s = consts2.tile([128, 4, C], F32R, tag="wpr")
            nc.sync.dma_start(
                out=w_pr_s, in_=w_pr.rearrange("(kc p) n -> p kc n", p=128)
            )
            mask_s = consts2.tile([128, 128], F32, tag="mask")
            nc.sync.dma_start(out=mask_s, in_=mask128[:, :])

            for j in range(NJ):
                prhs = prhs_pool.tile([128, 4, 512], F32R, tag="prhs")
                nkc = 4 * j + 4
                for t in range(4):  # head pairs (2t, 2t+1)
                    outs = [
                        psum_o.tile([128, 512], F32, tag=f"out{ab}",
                                    name=f"out{ab}")
                        for ab in range(2)
                    ]
                    for kc in range(nkc):
                        c = kc - 4 * j  # >=0 -> diagonal band tile
                        lo = 128 * c if c > 0 else 0
                        st = psum_s.tile([128, 2, 512], F32, tag="sab")
                        for ab in range(2):
                            part = slice(ab * 64, ab * 64 + 64)
                            nc.tensor.matmul(
                                st[:, ab, lo:],
                                lhsT=kT_s[part, t, kc * 128 : (kc + 1) * 128],
                                rhs=qT_s[part, t, j * 512 + lo : (j + 1) * 512],
                                start=True,
                                stop=True,
                                tile_position=(ab * 64, 0),
                            )
                        if c >= 0:
                            nc.vector.tensor_add(
                                st[:, :, lo : lo + 128],
                                st[:, :, lo : lo + 128],
                                bcast(mask_s[:, :], 2, 1),
                            )
                        pt = ppool.tile([128, 2, 512], F32R, tag="pab")
                        nc.scalar.activation(
                            pt[:, :, lo:], st[:, :, lo:], AFT.Exp
                        )
                        for ab in range(2):
                            nc.tensor.matmul(
                                outs[ab][0 : HD + 1, lo:],
                                lhsT=v_s[:, kc, 2 * t + ab, :],
                                rhs=pt[:, ab, lo:],
                                start=(kc == 0),
                                stop=(kc == nkc - 1),
                            )
                    for ab in range(2):
                        part = slice(ab * 64, ab * 64 + 64)
                        nc.vector.tensor_copy(prhs[part, t, :], outs[ab][0:HD, :])
                        r_h = rpool.tile([1, 512], F32, tag="rh",
                                         name="rh", bufs=4)
                        nc.vector.tensor_copy(r_h, outs[ab][HD : HD + 1, :])
                        nc.sync.dma_start(
                            out=r_bounce[j, 2 * t + ab], in_=r_h
                        )
                # normalize: r^-1 broadcast along the 64 hd-partitions of
                # each head (DRAM bounce for the partition-broadcast)
                rb = rpool.tile([128, 4, 512], F32, tag="rb")
                for h in range(HPG):
                    src = r_bounce[j, h, :]
                    bsrc = bass.AP(
                        tensor=src.tensor,
                        offset=src.offset,
                        ap=[[0, 64]] + list(src.ap),
                    )
                    nc.sync.dma_start(
                        out=rb[(h % 2) * 64 : (h % 2) * 64 + 64, h // 2, :],
                        in_=bsrc,
                    )
                nc.vector.reciprocal_approx_fast(out=rb, in_=rb)
                for t in range(4):
                    nc.vector.tensor_mul(prhs[:, t, :], prhs[:, t, :], rb[:, t, :])
                # output projection for this q-chunk
                for mp in range(8):
                    ps = psum_y.tile([128, 512], F32, tag="psy")
                    for kcp in range(4):
                        nc.tensor.matmul(
                            ps,
                            lhsT=w_pr_s[:, kcp, mp * 128 : (mp + 1) * 128],
                            rhs=prhs[:, kcp, :],
                            start=(kcp == 0),
                            stop=(kcp == 3),
                        )
                    yb = ypool.tile([128, 512], F32, tag="yb")
                    nc.vector.tensor_copy(yb, ps)
                    nc.sync.dma_start(
                        out=yT[mp * 128 : (mp + 1) * 128,
                               j * 512 : (j + 1) * 512],
                        in_=yb,
                    )
    return nc


def round_fp32r(a):
    """Round fp32 to fp32r (11-bit mantissa), matching walrus fp32_to_fp32r."""
    a = np.ascontiguousarray(a, dtype=np.float32)
    bits = a.view(np.uint32)
    out = ((bits.astype(np.uint64) + 0x800) & 0xFFFFF000).astype(np.uint32)
    return out.view(np.float32)


def make_core_inputs(x, w_attn, b_attn, w_proj):
    """Per-core input dicts; core index = batch*2 + head_group."""
    scale = np.float32(1.0 / np.sqrt(HD))
    mask = np.where(
        np.arange(128)[:, None] <= np.arange(128)[None, :], 0.0, MASK_NEG
    ).astype(np.float32)
    in_maps = []
    for b in range(NB):
        xTb = round_fp32r(x[b].T)
        for g in range(2):
            cols = slice(g * GC, (g + 1) * GC)
            w_qk = np.concatenate(
                [w_attn[:, cols] * scale, w_attn[:, C:][:, cols]], axis=1
            ).astype(np.float32)
            in_maps.append(
                {
                    "xT": xTb,
                    "w_qk": round_fp32r(w_qk),
                    "w_v": round_fp32r(w_attn[:, 2 * C :][:, cols]),
                    "b_q": np.ascontiguousarray(
                        b_attn[cols] * scale, dtype=np.float32
                    ),
                    "w_pr": round_fp32r(w_proj[cols, :]),
                    "mask128": mask,
                }
            )
    return in_maps


_NC_CACHE = None


def get_nc():
    global _NC_CACHE
    if _NC_CACHE is None:
        nc = bacc.Bacc("TRN2", target_bir_lowering=False, debug=False)
        build_kernel(nc)
        nc.compile()
        _NC_CACHE = nc
    return _NC_CACHE


def kernel(x, w_attn, b_attn, w_proj, b_proj, _want_trace=False):
    x = np.asarray(x, dtype=np.float32)
    w_attn = np.asarray(w_attn, dtype=np.float32)
    b_attn = np.asarray(b_attn, dtype=np.float32)
    w_proj = np.asarray(w_proj, dtype=np.float32)
    b_proj = np.asarray(b_proj, dtype=np.float32)

    nc = get_nc()
    in_maps = make_core_inputs(x, w_attn, b_attn, w_proj)
    res = run_bass_kernel_spmd(
        nc, in_maps, list(range(8)), trace=_want_trace
    )
    bias_total = (b_attn[2 * C :] @ w_proj + b_proj).astype(np.float32)
    out = np.empty((NB, T, C), np.float32)
    for b in range(NB):
        out[b] = (
            res.results[2 * b]["yT"].T
            + res.results[2 * b + 1]["yT"].T
            + bias_total[None, :]
        )
    if _want_trace:
        return out, res
    return out
